# revision 19
# baseline (speedup 1.0000x reference)
"""Two-layer GraphSAGE on 8 Trainium2 NeuronCores.

Sharding: nodes row-sharded across the 8 cores (12,500 each, padded to
12,800 = 100*128); edges partitioned by destination owner so the
segment-sum is core-local; weight matrices replicated.

Key structure (v2):
  * The gather table (all-gathered activations) is split into Q=4 chunks;
    each chunk is produced by its own AllGather so collectives overlap
    with compute: layer-1 chunk-b gathers only wait on x-chunk-b's
    AllGather, and layer-2's AllGather of h-chunk-q fires as soon as
    layer 1 finishes dst-quarter q (quarter-major scheduling).
  * Within a layer: for each dst quarter (25 windows), for each src
    chunk b, dma_gather pulls the per-edge source rows (int16 indices
    into chunk b) and one-hot matmuls accumulate into 25 PSUM-resident
    [128,128] f32 window accumulators (start/stop chained across the 4
    chunk passes).  Epilogue per window: 1/deg scale, transpose, the two
    SAGE matmuls, bias(+ReLU), write-back of the next layer's rows.
  * dma_gather calls cover up to 2048 edges (vs 1024) to amortize the
    ~1us fixed Q7 descriptor-gen cost; calls rotate over 4 SWDGE queues.
"""

import math
import os
import sys

import numpy as np

for _p in ("/opt/trn_rl_repo", "/root/.axon_site/_ro/trn_rl_repo"):
    if os.path.isdir(_p) and _p not in sys.path:
        sys.path.append(_p)

import concourse.bass as bass
import concourse.bacc as bacc
import concourse.tile as tile
from concourse import mybir

AG_END = os.environ.get("K_AG_END", "0") == "1" 
from concourse.masks import make_identity
from concourse.bass import _add_dep_helper

F32 = mybir.dt.float32
BF16 = mybir.dt.bfloat16
I16 = mybir.dt.int16
P = 128
PAD_DLOC = 999.0  # one-hot compare target for padding edges -> all-zero row


class Cfg:
    def __init__(self, N=100000, E=1600000, C=8, d=128, n_cls=40,
                 Q=4, MAX_IDX=1024, SINGLE_PACKET=True):
        self.N, self.E, self.C, self.d, self.n_cls = N, E, C, d, n_cls
        self.Q, self.MAX_IDX = Q, MAX_IDX
        self.SINGLE_PACKET = SINGLE_PACKET
        self.SH = N // C                       # real nodes per core (12500)
        self.SHP = 12800                       # padded shard rows (100*128)
        self.W = self.SHP // P                 # dst windows per core (100)
        self.WQ = self.W // Q                  # windows per dst chunk (25)
        self.WG = 5                            # windows per PSUM group
        self.NG = self.W // self.WG            # groups (20)
        self.CH_SH = self.SHP // Q             # shard rows per chunk (3200)
        self.CH_TBL = C * self.CH_SH           # table rows per chunk (25600)
        self.TBL = Q * self.CH_TBL             # gather-table rows (102400)
        self.MAXB = MAX_IDX // P               # blocks per gather call (16)
        assert d == P, "feature dim must be 128"
        assert self.CH_TBL <= 32768, "chunk must fit int16 indices"
        assert self.W % Q == 0
        assert self.WQ % self.WG == 0          # groups align with chunks


class Schedule:
    """Core-independent loop structure + per-core gather/one-hot data."""
    pass


def build_schedule(cfg: Cfg, src: np.ndarray, dst: np.ndarray,
                   deg: np.ndarray) -> Schedule:
    C, W, Q, WQ = cfg.C, cfg.W, cfg.Q, cfg.WQ
    SH, SHP, CH_SH, CH_TBL = cfg.SH, cfg.SHP, cfg.CH_SH, cfg.CH_TBL

    owner = src // SH
    within = src - owner * SH                  # 0..SH-1
    b_e = within // CH_SH                      # src chunk (= bucket)
    trow = b_e * CH_TBL + owner * CH_SH + (within - b_e * CH_SH)
    loc16 = (trow - b_e * CH_TBL).astype(np.int16)   # 0..CH_TBL-1

    dcore = dst // SH
    dloc = dst - dcore * SH
    w_e = dloc // P                            # dst window within the core
    WG, NG = cfg.WG, cfg.NG
    g_e = w_e // WG                            # dst PSUM group
    dwin = (dloc % P).astype(np.float32)       # one-hot target

    # stream order per core: group -> bucket -> window -> block
    key = ((dcore * NG + g_e) * Q + b_e) * W + w_e
    cnt = np.zeros((C, W, Q), np.int64)
    np.add.at(cnt, (dcore, w_e, b_e), 1)

    M = (np.ceil(cnt.max(axis=0) / P)).astype(np.int64)        # [W, Q]
    for w in range(W):                                         # >=1 block
        if M[w].sum() == 0:
            M[w, 0] = 1

    s = Schedule()
    s.M = M
    s.tot_blocks_w = M.sum(axis=1)             # matmul count per window
    s.runs = []                                # [g][b] -> (blk0, nblk, wblocks)
    blkoff = 0
    blk_of = np.zeros((W, Q), np.int64)
    blockwin = []
    for g in range(NG):
        wins = list(range(g * WG, (g + 1) * WG))
        runs_b = []
        for b in range(Q):
            wblocks = [(w, int(M[w, b])) for w in wins if M[w, b] > 0]
            nblk = sum(m for _, m in wblocks)
            off = blkoff
            for w, m in wblocks:
                blk_of[w, b] = off
                blockwin.extend([w] * m)
                off += m
            runs_b.append((b, blkoff, nblk, wblocks))
            blkoff += nblk
        s.runs.append((g, wins, runs_b))
    s.B_tot = blkoff
    s.T_idx = s.B_tot * P
    s.blockwin = np.array(blockwin, np.int64)

    # per-edge stream positions
    order = np.argsort(key, kind="stable")
    koff = np.zeros(C * NG * Q * W + 1, np.int64)
    kcnt = np.bincount(key, minlength=C * NG * Q * W)
    np.cumsum(kcnt, out=koff[1:])
    rank = np.arange(cfg.E, dtype=np.int64) - koff[key[order]]
    pos = blk_of[w_e[order], b_e[order]] * P + rank

    idx16 = np.zeros((C, 128, s.T_idx // 16), np.int16)
    dstloc = np.full((C, 128, s.B_tot), PAD_DLOC, np.float32)
    for c in range(C):
        m = dcore[order] == c
        p_c = pos[m]
        flat = np.zeros(s.T_idx, np.int16)
        flat[p_c] = loc16[order][m]
        wrapped = flat.reshape(-1, 16).T       # [16, T/16]
        idx16[c] = np.tile(wrapped, (8, 1))    # replicate for 8 Q7 cores
        dl = np.full(s.B_tot * P, PAD_DLOC, np.float32)
        dl[p_c] = dwin[order][m]
        dstloc[c] = dl.reshape(s.B_tot, P).T   # [128 lanes, B_tot blocks]
    s.idx16, s.dstloc = idx16, dstloc

    invdeg = 1.0 / np.maximum(deg, 1.0)
    inv = np.ones((C, 128, W), np.float32)
    for c in range(C):
        v = np.ones(SHP, np.float32)
        v[:SH] = invdeg[c * SH:(c + 1) * SH]
        inv[c] = v.reshape(W, P).T
    s.invdeg_t = inv
    return s


def build_program(cfg: Cfg, s: Schedule, debug: bool = False):
    """Emit the SPMD Bass program (identical on all 8 cores)."""
    C, W, Q, WQ, NCLS = cfg.C, cfg.W, cfg.Q, cfg.WQ, cfg.n_cls
    SHP, TBL, CH_SH, CH_TBL = cfg.SHP, cfg.TBL, cfg.CH_SH, cfg.CH_TBL
    MAXB = cfg.MAXB

    nc = bacc.Bacc("TRN2", target_bir_lowering=False, debug=debug,
                   num_devices=C, num_swdge_queues=4,
                   dynamic_dma_scratch_size=65536)

    x_own = nc.dram_tensor("x_own", [SHP, P], F32, kind="ExternalInput")
    idx_in = nc.dram_tensor("idx16", [128, s.T_idx // 16], I16,
                            kind="ExternalInput")
    dloc_in = nc.dram_tensor("dstloc", [128, s.B_tot], F32,
                             kind="ExternalInput")
    inv_in = nc.dram_tensor("invdeg", [128, W], F32, kind="ExternalInput")
    iota_in = nc.dram_tensor("iota", [128, 128], F32, kind="ExternalInput")
    w_ins = {}
    for nm, shp in (("wl1t", [P, P]), ("wr1t", [P, P]),
                    ("wl2t", [P, NCLS]), ("wr2t", [P, NCLS])):
        w_ins[nm] = nc.dram_tensor(nm, shp, F32, kind="ExternalInput")
    bl1_in = nc.dram_tensor("bl1", [P, 1], F32, kind="ExternalInput")
    bl2_in = nc.dram_tensor("bl2", [NCLS, 1], F32, kind="ExternalInput")
    out_d = nc.dram_tensor("out", [SHP, NCLS], F32, kind="ExternalOutput")

    x_in_c = [nc.dram_tensor(f"x_bf_own{q}", [CH_SH, P], BF16)
              for q in range(Q)]
    h_in_c = [nc.dram_tensor(f"h_bf_own{q}", [CH_SH, P], BF16)
              for q in range(Q)]
    x_full_c = [nc.dram_tensor(f"x_full{q}", [CH_TBL, P], BF16,
                               addr_space="Shared") for q in range(Q)]
    h_full_c = [nc.dram_tensor(f"h_full{q}", [CH_TBL, P], BF16,
                               addr_space="Shared") for q in range(Q)]

    rg = [list(range(C))]
    last_cc = [None]

    def issue_ag(in_ap, out_ap):
        cc = nc.gpsimd.collective_compute(
            "AllGather", mybir.AluOpType.bypass, replica_groups=rg,
            ins=[in_ap], outs=[out_ap])
        last_cc[0] = cc
        return cc

    with tile.TileContext(nc) as tc:
        cpool = tc.alloc_tile_pool(name="consts", bufs=1)
        stage = tc.alloc_tile_pool(name="stage", bufs=2)

        ident_b = cpool.tile([P, P], BF16)
        make_identity(nc, ident_b[:])
        ident_f = cpool.tile([P, P], F32)
        make_identity(nc, ident_f[:])

        iota_f = cpool.tile([128, 128], F32)
        nc.sync.dma_start(out=iota_f[:], in_=iota_in[:])
        iota_b = cpool.tile([128, 128], BF16)
        nc.vector.tensor_copy(out=iota_b[:], in_=iota_f[:])

        wt = {}
        for nm in ("wl1t", "wr1t", "wl2t", "wr2t"):
            shp = [P, P] if nm in ("wl1t", "wr1t") else [P, NCLS]
            st = stage.tile(shp, F32, tag="wstage")
            nc.sync.dma_start(out=st[:], in_=w_ins[nm][:])
            wt[nm] = cpool.tile(shp, BF16, name=f"w_{nm}")
            nc.vector.tensor_copy(out=wt[nm][:], in_=st[:])
        bl1_t = cpool.tile([P, 1], F32)
        nc.sync.dma_start(out=bl1_t[:], in_=bl1_in[:])
        bl2_t = cpool.tile([NCLS, 1], F32)
        nc.sync.dma_start(out=bl2_t[:], in_=bl2_in[:])
        inv_t = cpool.tile([128, W], F32)
        nc.sync.dma_start(out=inv_t[:], in_=inv_in[:])
        idx_sb = cpool.tile([128, s.T_idx // 16], I16)
        nc.sync.dma_start(out=idx_sb[:], in_=idx_in[:])
        dloc_f = stage.tile([128, s.B_tot], F32, tag="dlocf", bufs=1)
        nc.sync.dma_start(out=dloc_f[:], in_=dloc_in[:])
        dloc_sb = cpool.tile([128, s.B_tot], BF16)
        nc.vector.tensor_copy(out=dloc_sb[:], in_=dloc_f[:])

        xT = cpool.tile([P, SHP], BF16)        # x_own^T, bf16
        hT = cpool.tile([P, SHP], BF16)        # h_own^T, bf16

        # ---- phase 0: cast x to bf16 chunk by chunk; AllGather each chunk
        # as soon as its rows are written so collectives overlap compute.
        BW = 5                                 # windows per phase-0 batch
        with tc.tile_pool(name="ph0", bufs=3) as ph0, \
             tc.tile_pool(name="ph0p", bufs=2, space="PSUM") as ph0p:
            for q in range(Q):
                for w0 in range(q * WQ, (q + 1) * WQ, BW):
                    r0 = w0 * P
                    c0 = r0 - q * CH_SH
                    xin = x_own[r0:r0 + BW * P, :].rearrange(
                        "(w p) f -> p w f", p=P)
                    xrow_f = ph0.tile([P, BW, P], F32, tag="xf")
                    nc.sync.dma_start(out=xrow_f[:], in_=xin)
                    xrow_b = ph0.tile([P, BW, P], BF16, tag="xb")
                    nc.vector.tensor_copy(out=xrow_b[:], in_=xrow_f[:])
                    nc.sync.dma_start(
                        out=x_in_c[q][c0:c0 + BW * P, :].rearrange(
                            "(w p) f -> p w f", p=P),
                        in_=xrow_b[:])
                    pt = ph0p.tile([P, BW, P], BF16, tag="pt")
                    for j in range(BW):
                        nc.tensor.transpose(out=pt[:, j, :],
                                            in_=xrow_b[:, j, :],
                                            identity=ident_b[:])
                    nc.vector.tensor_copy(
                        out=xT[:, r0:r0 + BW * P].rearrange(
                            "p (w f) -> p w f", w=BW),
                        in_=pt[:])
                if not AG_END:
                    issue_ag(x_in_c[q][:], x_full_c[q][:])
            if AG_END:
                for q2 in range(Q):
                    issue_ag(x_in_c[q2][:], x_full_c[q2][:])

        qctr = [0]

        def sage_layer(table, dense_rhs, wl, wr, bias_t, m_out, out_sink,
                       on_chunk_done=None):
            """One SAGE conv over the edge schedule (group-major)."""
            gp = tc.alloc_tile_pool(name="gath", bufs=10)
            ohp = tc.alloc_tile_pool(name="oh", bufs=10)
            ap_ = tc.alloc_tile_pool(name="psA", bufs=cfg.WG, space="PSUM")
            ep_ = tc.alloc_tile_pool(name="psE", bufs=1, space="PSUM")
            sb_ = tc.alloc_tile_pool(name="esb", bufs=3)
            gpc = cfg.WQ // cfg.WG             # groups per dst chunk
            done_w = np.zeros(W, np.int64)
            for g, wins, runs_b in s.runs:
                psA = {w: ap_.tile([P, P], F32, tag="A",
                                   name=f"psA_{g}_{w}")
                       for w in wins}
                for b, blk0, nblk, wblocks in runs_b:
                    if nblk == 0:
                        continue
                    for c0 in range(0, nblk, MAXB):
                        cn = min(MAXB, nblk - c0)
                        gt = gp.tile([128, MAXB, P], BF16, tag="g")
                        i0 = (blk0 + c0) * P // 16
                        nc.gpsimd.dma_gather(
                            out_ap=gt[:, :cn, :],
                            in_ap=table[b][:],
                            idxs_ap=idx_sb[:, i0:i0 + cn * P // 16],
                            num_idxs=cn * P,
                            num_idxs_reg=cn * P,
                            elem_size=P,
                            single_packet=cfg.SINGLE_PACKET,
                            queue_num=qctr[0] % 4)
                        qctr[0] += 1
                        oht = ohp.tile([128, MAXB, P], BF16, tag="oh")
                        nc.vector.tensor_tensor(
                            out=oht[:, :cn, :],
                            in0=iota_b[:].rearrange(
                                "p (o n) -> p o n", o=1).to_broadcast(
                                [128, cn, P]),
                            in1=dloc_sb[:, blk0 + c0:blk0 + c0 + cn].rearrange(
                                "p (n o) -> p n o", o=1).to_broadcast(
                                [128, cn, P]),
                            op=mybir.AluOpType.is_equal)
                        for j in range(cn):
                            w = int(s.blockwin[blk0 + c0 + j])
                            nc.tensor.matmul(
                                psA[w][:], lhsT=oht[:, j, :],
                                rhs=gt[:, j, :],
                                start=(done_w[w] == 0),
                                stop=(done_w[w] == s.tot_blocks_w[w] - 1))
                            done_w[w] += 1
                # epilogue per window of this quarter
                for w in wins:
                    wc = w * P
                    agg = sb_.tile([P, P], BF16, tag="agg")
                    nc.scalar.mul(agg[:], psA[w][:], inv_t[:, w:w + 1])
                    pt = ep_.tile([P, P], BF16, tag="T")
                    nc.tensor.transpose(out=pt[:], in_=agg[:],
                                        identity=ident_b[:])
                    aggT = sb_.tile([P, P], BF16, tag="aggT")
                    nc.vector.tensor_copy(out=aggT[:], in_=pt[:])
                    pb = ep_.tile([m_out, P], F32, tag="B")
                    nc.tensor.matmul(pb[:], lhsT=wl[:], rhs=aggT[:],
                                     start=True, stop=False)
                    nc.tensor.matmul(pb[:], lhsT=wr[:],
                                     rhs=dense_rhs[:, wc:wc + P],
                                     start=False, stop=True)
                    out_sink(w, pb, bias_t)
                if on_chunk_done is not None and (g + 1) % gpc == 0:
                    on_chunk_done(g // gpc)
            for pool in (sb_, ep_, ap_, ohp, gp):
                pool.release()

        # ---- layer 1 ----
        with tc.tile_pool(name="l1o", bufs=3) as l1o, \
             tc.tile_pool(name="l1p", bufs=1, space="PSUM") as l1p:
            def sink1(w, pb, bias_t):
                wc = w * P
                nc.scalar.activation(hT[:, wc:wc + P], pb[:],
                                     mybir.ActivationFunctionType.Relu,
                                     bias=bias_t[:], scale=1.0)
                pc = l1p.tile([P, P], BF16, tag="C")
                nc.tensor.transpose(out=pc[:], in_=hT[:, wc:wc + P],
                                    identity=ident_b[:])
                hrow = l1o.tile([P, P], BF16, tag="hrow")
                nc.vector.tensor_copy(out=hrow[:], in_=pc[:])
                q = w // WQ
                c0 = wc - q * CH_SH
                nc.sync.dma_start(out=h_in_c[q][c0:c0 + P, :], in_=hrow[:])

            def chunk_done1(q):
                if not AG_END:
                    issue_ag(h_in_c[q][:], h_full_c[q][:])

            sage_layer(x_full_c, xT, wt["wl1t"], wt["wr1t"], bl1_t,
                       m_out=P, out_sink=sink1,
                       on_chunk_done=chunk_done1)
            if AG_END:
                for q2 in range(Q):
                    issue_ag(h_in_c[q2][:], h_full_c[q2][:])

        # ---- layer 2 ----
        with tc.tile_pool(name="l2o", bufs=3) as l2o, \
             tc.tile_pool(name="l2p", bufs=1, space="PSUM") as l2p:
            def sink2(w, pb, bias_t):
                wc = w * P
                oT = l2o.tile([NCLS, P], F32, tag="oT")
                nc.scalar.activation(oT[:], pb[:],
                                     mybir.ActivationFunctionType.Identity,
                                     bias=bias_t[:], scale=1.0)
                pc = l2p.tile([P, NCLS], F32, tag="C2")
                nc.tensor.matmul(pc[:], lhsT=oT[:], rhs=ident_f[:NCLS, :NCLS],
                                 is_transpose=True)
                orow = l2o.tile([P, NCLS], F32, tag="orow")
                nc.vector.tensor_copy(out=orow[:], in_=pc[:])
                nc.sync.dma_start(out=out_d[wc:wc + P, :], in_=orow[:])

            sage_layer(h_full_c, hT, wt["wl2t"], wt["wr2t"], bl2_t,
                       m_out=NCLS, out_sink=sink2)

        for pool in (stage, cpool):
            pool.release()

    nc.compile()
    return nc


def make_inputs(cfg: Cfg, s: Schedule, x, Wl1, bl1, Wr1, Wl2, bl2, Wr2):
    """Per-core in_maps for run_bass_kernel_spmd."""
    C, SH, SHP, W, NCLS = cfg.C, cfg.SH, cfg.SHP, cfg.W, cfg.n_cls
    iota = np.tile(np.arange(128, dtype=np.float32), (128, 1))
    maps = []
    for c in range(C):
        xo = np.zeros((SHP, P), np.float32)
        xo[:SH] = x[c * SH:(c + 1) * SH]
        maps.append({
            "x_own": xo,
            "idx16": s.idx16[c],
            "dstloc": s.dstloc[c],
            "invdeg": s.invdeg_t[c],
            "iota": iota,
            "wl1t": np.ascontiguousarray(Wl1.T.astype(np.float32)),
            "wr1t": np.ascontiguousarray(Wr1.T.astype(np.float32)),
            "wl2t": np.ascontiguousarray(Wl2.T.astype(np.float32)),
            "wr2t": np.ascontiguousarray(Wr2.T.astype(np.float32)),
            "bl1": bl1.astype(np.float32).reshape(P, 1),
            "bl2": bl2.astype(np.float32).reshape(NCLS, 1),
        })
    return maps


def prepare(cfg: Cfg, x, edge_index, Wl1, bl1, Wr1, Wl2, bl2, Wr2):
    x = np.asarray(x, np.float32)
    ei = np.asarray(edge_index, np.int64)
    src, dst = ei[0], ei[1]
    deg = np.bincount(dst, minlength=cfg.N).astype(np.float32)
    s = build_schedule(cfg, src, dst, deg)
    maps = make_inputs(cfg, s, x, Wl1, bl1, Wr1, Wl2, bl2, Wr2)
    return s, maps


def run(x, edge_index, Wl1, bl1, Wr1, Wl2, bl2, Wr2, cfg=None, **spmd_kwargs):
    from concourse.bass_utils import run_bass_kernel_spmd
    cfg = cfg or Cfg(MAX_IDX=int(os.environ.get("K_MAX_IDX", "1024")))  # 2048 overflows the SWDGE ring -> device hang
    s, maps = prepare(cfg, x, edge_index, Wl1, bl1, Wr1, Wl2, bl2, Wr2)
    nc = build_program(cfg, s)
    res = run_bass_kernel_spmd(nc, maps, core_ids=list(range(cfg.C)),
                               **spmd_kwargs)
    out = np.concatenate([res.results[c]["out"][:cfg.SH]
                          for c in range(cfg.C)], axis=0)
    return out.astype(np.float32), res


def kernel(x, edge_index, Wl1, bl1, Wr1, Wl2, bl2, Wr2):
    out, _ = run(x, edge_index, Wl1, bl1, Wr1, Wl2, bl2, Wr2)
    return out


# revision 21
# speedup vs baseline: 1.0561x; 1.0561x over previous
"""Two-layer GraphSAGE on 8 Trainium2 NeuronCores.

Sharding: nodes row-sharded across the 8 cores (12,500 each, padded to
12,800 = 100*128); edges partitioned by destination owner so the
segment-sum is core-local; weight matrices replicated.

Key structure (v2):
  * The gather table (all-gathered activations) is split into Q=4 chunks;
    each chunk is produced by its own AllGather so collectives overlap
    with compute: layer-1 chunk-b gathers only wait on x-chunk-b's
    AllGather, and layer-2's AllGather of h-chunk-q fires as soon as
    layer 1 finishes dst-quarter q (quarter-major scheduling).
  * Within a layer: for each dst quarter (25 windows), for each src
    chunk b, dma_gather pulls the per-edge source rows (int16 indices
    into chunk b) and one-hot matmuls accumulate into 25 PSUM-resident
    [128,128] f32 window accumulators (start/stop chained across the 4
    chunk passes).  Epilogue per window: 1/deg scale, transpose, the two
    SAGE matmuls, bias(+ReLU), write-back of the next layer's rows.
  * dma_gather calls cover up to 2048 edges (vs 1024) to amortize the
    ~1us fixed Q7 descriptor-gen cost; calls rotate over 4 SWDGE queues.
"""

import math
import os
import sys

import numpy as np

for _p in ("/opt/trn_rl_repo", "/root/.axon_site/_ro/trn_rl_repo"):
    if os.path.isdir(_p) and _p not in sys.path:
        sys.path.append(_p)

import concourse.bass as bass
import concourse.bacc as bacc
import concourse.tile as tile
from concourse import mybir

AG_END = os.environ.get("K_AG_END", "0") == "1" 
from concourse.masks import make_identity
from concourse.bass import _add_dep_helper

F32 = mybir.dt.float32
BF16 = mybir.dt.bfloat16
I16 = mybir.dt.int16
P = 128
PAD_DLOC = 999.0  # one-hot compare target for padding edges -> all-zero row


class Cfg:
    def __init__(self, N=100000, E=1600000, C=8, d=128, n_cls=40,
                 Q=4, MAX_IDX=1024, SINGLE_PACKET=True):
        self.N, self.E, self.C, self.d, self.n_cls = N, E, C, d, n_cls
        self.Q, self.MAX_IDX = Q, MAX_IDX
        self.SINGLE_PACKET = SINGLE_PACKET
        self.SH = N // C                       # real nodes per core (12500)
        self.SHP = 12800                       # padded shard rows (100*128)
        self.W = self.SHP // P                 # dst windows per core (100)
        self.WQ = self.W // Q                  # windows per dst chunk (25)
        self.WG = 5                            # windows per PSUM group
        self.NG = self.W // self.WG            # groups (20)
        self.CH_SH = self.SHP // Q             # shard rows per chunk (3200)
        self.CH_TBL = C * self.CH_SH           # table rows per chunk (25600)
        self.TBL = Q * self.CH_TBL             # gather-table rows (102400)
        self.MAXB = MAX_IDX // P               # blocks per gather call (16)
        assert d == P, "feature dim must be 128"
        assert self.CH_TBL <= 32768, "chunk must fit int16 indices"
        assert self.W % Q == 0
        assert self.WQ % self.WG == 0          # groups align with chunks


class Schedule:
    """Core-independent loop structure + per-core gather/one-hot data."""
    pass


def build_schedule(cfg: Cfg, src: np.ndarray, dst: np.ndarray,
                   deg: np.ndarray) -> Schedule:
    C, W, Q, WQ = cfg.C, cfg.W, cfg.Q, cfg.WQ
    SH, SHP, CH_SH, CH_TBL = cfg.SH, cfg.SHP, cfg.CH_SH, cfg.CH_TBL

    owner = src // SH
    within = src - owner * SH                  # 0..SH-1
    b_e = within // CH_SH                      # src chunk (= bucket)
    trow = b_e * CH_TBL + owner * CH_SH + (within - b_e * CH_SH)
    loc16 = (trow - b_e * CH_TBL).astype(np.int16)   # 0..CH_TBL-1

    dcore = dst // SH
    dloc = dst - dcore * SH
    w_e = dloc // P                            # dst window within the core
    WG, NG = cfg.WG, cfg.NG
    g_e = w_e // WG                            # dst PSUM group
    dwin = (dloc % P).astype(np.float32)       # one-hot target

    # stream order per core: group -> bucket -> window -> block
    key = ((dcore * NG + g_e) * Q + b_e) * W + w_e
    cnt = np.zeros((C, W, Q), np.int64)
    np.add.at(cnt, (dcore, w_e, b_e), 1)

    M = (np.ceil(cnt.max(axis=0) / P)).astype(np.int64)        # [W, Q]
    for w in range(W):                                         # >=1 block
        if M[w].sum() == 0:
            M[w, 0] = 1

    s = Schedule()
    s.M = M
    s.tot_blocks_w = M.sum(axis=1)             # matmul count per window
    s.runs = []                                # [g][b] -> (blk0, nblk, wblocks)
    blkoff = 0
    blk_of = np.zeros((W, Q), np.int64)
    blockwin = []
    for g in range(NG):
        wins = list(range(g * WG, (g + 1) * WG))
        runs_b = []
        for b in range(Q):
            wblocks = [(w, int(M[w, b])) for w in wins if M[w, b] > 0]
            nblk = sum(m for _, m in wblocks)
            off = blkoff
            for w, m in wblocks:
                blk_of[w, b] = off
                blockwin.extend([w] * m)
                off += m
            runs_b.append((b, blkoff, nblk, wblocks))
            blkoff += nblk
        s.runs.append((g, wins, runs_b))
    s.B_tot = blkoff
    s.T_idx = s.B_tot * P
    s.blockwin = np.array(blockwin, np.int64)

    # per-edge stream positions
    order = np.argsort(key, kind="stable")
    koff = np.zeros(C * NG * Q * W + 1, np.int64)
    kcnt = np.bincount(key, minlength=C * NG * Q * W)
    np.cumsum(kcnt, out=koff[1:])
    rank = np.arange(cfg.E, dtype=np.int64) - koff[key[order]]
    pos = blk_of[w_e[order], b_e[order]] * P + rank

    idx16 = np.zeros((C, 128, s.T_idx // 16), np.int16)
    dstloc = np.full((C, 128, s.B_tot), PAD_DLOC, np.float32)
    for c in range(C):
        m = dcore[order] == c
        p_c = pos[m]
        flat = np.zeros(s.T_idx, np.int16)
        flat[p_c] = loc16[order][m]
        wrapped = flat.reshape(-1, 16).T       # [16, T/16]
        idx16[c] = np.tile(wrapped, (8, 1))    # replicate for 8 Q7 cores
        dl = np.full(s.B_tot * P, PAD_DLOC, np.float32)
        dl[p_c] = dwin[order][m]
        dstloc[c] = dl.reshape(s.B_tot, P).T   # [128 lanes, B_tot blocks]
    s.idx16, s.dstloc = idx16, dstloc

    invdeg = 1.0 / np.maximum(deg, 1.0)
    inv = np.ones((C, 128, W), np.float32)
    for c in range(C):
        v = np.ones(SHP, np.float32)
        v[:SH] = invdeg[c * SH:(c + 1) * SH]
        inv[c] = v.reshape(W, P).T
    s.invdeg_t = inv
    return s


def build_program(cfg: Cfg, s: Schedule, debug: bool = False):
    """Emit the SPMD Bass program (identical on all 8 cores)."""
    C, W, Q, WQ, NCLS = cfg.C, cfg.W, cfg.Q, cfg.WQ, cfg.n_cls
    SHP, TBL, CH_SH, CH_TBL = cfg.SHP, cfg.TBL, cfg.CH_SH, cfg.CH_TBL
    MAXB = cfg.MAXB

    nc = bacc.Bacc("TRN2", target_bir_lowering=False, debug=debug,
                   num_devices=C, num_swdge_queues=4,
                   dynamic_dma_scratch_size=65536)

    x_own = nc.dram_tensor("x_own", [SHP, P], F32, kind="ExternalInput")
    idx_in = nc.dram_tensor("idx16", [128, s.T_idx // 16], I16,
                            kind="ExternalInput")
    dloc_in = nc.dram_tensor("dstloc", [128, s.B_tot], F32,
                             kind="ExternalInput")
    inv_in = nc.dram_tensor("invdeg", [128, W], F32, kind="ExternalInput")
    iota_in = nc.dram_tensor("iota", [128, 128], F32, kind="ExternalInput")
    w_ins = {}
    for nm, shp in (("wl1t", [P, P]), ("wr1t", [P, P]),
                    ("wl2t", [P, NCLS]), ("wr2t", [P, NCLS])):
        w_ins[nm] = nc.dram_tensor(nm, shp, F32, kind="ExternalInput")
    bl1_in = nc.dram_tensor("bl1", [P, 1], F32, kind="ExternalInput")
    bl2_in = nc.dram_tensor("bl2", [NCLS, 1], F32, kind="ExternalInput")
    out_d = nc.dram_tensor("out", [SHP, NCLS], F32, kind="ExternalOutput")

    x_in_c = [nc.dram_tensor(f"x_bf_own{q}", [CH_SH, P], BF16)
              for q in range(Q)]
    h_in_c = [nc.dram_tensor(f"h_bf_own{q}", [CH_SH, P], BF16)
              for q in range(Q)]
    x_full_c = [nc.dram_tensor(f"x_full{q}", [CH_TBL, P], BF16,
                               addr_space="Shared") for q in range(Q)]
    h_full_c = [nc.dram_tensor(f"h_full{q}", [CH_TBL, P], BF16,
                               addr_space="Shared") for q in range(Q)]

    rg = [list(range(C))]
    last_cc = [None]

    def issue_ag(in_ap, out_ap):
        cc = nc.gpsimd.collective_compute(
            "AllGather", mybir.AluOpType.bypass, replica_groups=rg,
            ins=[in_ap], outs=[out_ap])
        if last_cc[0] is not None:
            # serialize collectives: at most one in flight on the fabric
            _add_dep_helper(cc.ins, last_cc[0].ins, sync=True,
                            reason="serialize collectives")
        last_cc[0] = cc
        return cc

    with tile.TileContext(nc) as tc:
        cpool = tc.alloc_tile_pool(name="consts", bufs=1)
        stage = tc.alloc_tile_pool(name="stage", bufs=2)

        ident_b = cpool.tile([P, P], BF16)
        make_identity(nc, ident_b[:])
        ident_f = cpool.tile([P, P], F32)
        make_identity(nc, ident_f[:])

        iota_f = cpool.tile([128, 128], F32)
        nc.sync.dma_start(out=iota_f[:], in_=iota_in[:])
        iota_b = cpool.tile([128, 128], BF16)
        nc.vector.tensor_copy(out=iota_b[:], in_=iota_f[:])

        wt = {}
        for nm in ("wl1t", "wr1t", "wl2t", "wr2t"):
            shp = [P, P] if nm in ("wl1t", "wr1t") else [P, NCLS]
            st = stage.tile(shp, F32, tag="wstage")
            nc.sync.dma_start(out=st[:], in_=w_ins[nm][:])
            wt[nm] = cpool.tile(shp, BF16, name=f"w_{nm}")
            nc.vector.tensor_copy(out=wt[nm][:], in_=st[:])
        bl1_t = cpool.tile([P, 1], F32)
        nc.sync.dma_start(out=bl1_t[:], in_=bl1_in[:])
        bl2_t = cpool.tile([NCLS, 1], F32)
        nc.sync.dma_start(out=bl2_t[:], in_=bl2_in[:])
        inv_t = cpool.tile([128, W], F32)
        nc.sync.dma_start(out=inv_t[:], in_=inv_in[:])
        idx_sb = cpool.tile([128, s.T_idx // 16], I16)
        nc.sync.dma_start(out=idx_sb[:], in_=idx_in[:])
        dloc_f = stage.tile([128, s.B_tot], F32, tag="dlocf", bufs=1)
        nc.sync.dma_start(out=dloc_f[:], in_=dloc_in[:])
        dloc_sb = cpool.tile([128, s.B_tot], BF16)
        nc.vector.tensor_copy(out=dloc_sb[:], in_=dloc_f[:])

        xT = cpool.tile([P, SHP], BF16)        # x_own^T, bf16
        hT = cpool.tile([P, SHP], BF16)        # h_own^T, bf16

        # ---- phase 0: cast x to bf16 chunk by chunk; AllGather each chunk
        # as soon as its rows are written so collectives overlap compute.
        BW = 5                                 # windows per phase-0 batch
        with tc.tile_pool(name="ph0", bufs=3) as ph0, \
             tc.tile_pool(name="ph0p", bufs=2, space="PSUM") as ph0p:
            for q in range(Q):
                for w0 in range(q * WQ, (q + 1) * WQ, BW):
                    r0 = w0 * P
                    c0 = r0 - q * CH_SH
                    xin = x_own[r0:r0 + BW * P, :].rearrange(
                        "(w p) f -> p w f", p=P)
                    xrow_f = ph0.tile([P, BW, P], F32, tag="xf")
                    nc.sync.dma_start(out=xrow_f[:], in_=xin)
                    xrow_b = ph0.tile([P, BW, P], BF16, tag="xb")
                    nc.vector.tensor_copy(out=xrow_b[:], in_=xrow_f[:])
                    nc.sync.dma_start(
                        out=x_in_c[q][c0:c0 + BW * P, :].rearrange(
                            "(w p) f -> p w f", p=P),
                        in_=xrow_b[:])
                    pt = ph0p.tile([P, BW, P], BF16, tag="pt")
                    for j in range(BW):
                        nc.tensor.transpose(out=pt[:, j, :],
                                            in_=xrow_b[:, j, :],
                                            identity=ident_b[:])
                    nc.vector.tensor_copy(
                        out=xT[:, r0:r0 + BW * P].rearrange(
                            "p (w f) -> p w f", w=BW),
                        in_=pt[:])
                if not AG_END:
                    issue_ag(x_in_c[q][:], x_full_c[q][:])
            if AG_END:
                for q2 in range(Q):
                    issue_ag(x_in_c[q2][:], x_full_c[q2][:])

        qctr = [0]

        # pools shared by both layers: re-allocating them at the layer
        # boundary makes Tile drain the whole pipeline there.
        gp = tc.alloc_tile_pool(name="gath", bufs=8)
        ohp = tc.alloc_tile_pool(name="oh", bufs=8)
        ap_ = tc.alloc_tile_pool(name="psA", bufs=cfg.WG, space="PSUM")
        ep_ = tc.alloc_tile_pool(name="psE", bufs=1, space="PSUM")
        sb_ = tc.alloc_tile_pool(name="esb", bufs=3)
        skp = tc.alloc_tile_pool(name="psSink", bufs=1, space="PSUM")
        sko = tc.alloc_tile_pool(name="sinko", bufs=3)

        def sage_layer(table, dense_rhs, wl, wr, bias_t, m_out, out_sink,
                       on_chunk_done=None):
            """One SAGE conv over the edge schedule (group-major)."""
            gpc = cfg.WQ // cfg.WG             # groups per dst chunk
            done_w = np.zeros(W, np.int64)
            for g, wins, runs_b in s.runs:
                psA = {w: ap_.tile([P, P], F32, tag="A",
                                   name=f"psA_{g}_{w}")
                       for w in wins}
                for b, blk0, nblk, wblocks in runs_b:
                    if nblk == 0:
                        continue
                    for c0 in range(0, nblk, MAXB):
                        cn = min(MAXB, nblk - c0)
                        gt = gp.tile([128, MAXB, P], BF16, tag="g")
                        i0 = (blk0 + c0) * P // 16
                        nc.gpsimd.dma_gather(
                            out_ap=gt[:, :cn, :],
                            in_ap=table[b][:],
                            idxs_ap=idx_sb[:, i0:i0 + cn * P // 16],
                            num_idxs=cn * P,
                            num_idxs_reg=cn * P,
                            elem_size=P,
                            single_packet=cfg.SINGLE_PACKET,
                            queue_num=qctr[0] % 4)
                        qctr[0] += 1
                        oht = ohp.tile([128, MAXB, P], BF16, tag="oh")
                        nc.vector.tensor_tensor(
                            out=oht[:, :cn, :],
                            in0=iota_b[:].rearrange(
                                "p (o n) -> p o n", o=1).to_broadcast(
                                [128, cn, P]),
                            in1=dloc_sb[:, blk0 + c0:blk0 + c0 + cn].rearrange(
                                "p (n o) -> p n o", o=1).to_broadcast(
                                [128, cn, P]),
                            op=mybir.AluOpType.is_equal)
                        for j in range(cn):
                            w = int(s.blockwin[blk0 + c0 + j])
                            nc.tensor.matmul(
                                psA[w][:], lhsT=oht[:, j, :],
                                rhs=gt[:, j, :],
                                start=(done_w[w] == 0),
                                stop=(done_w[w] == s.tot_blocks_w[w] - 1))
                            done_w[w] += 1
                # epilogue per window of this quarter
                for w in wins:
                    wc = w * P
                    agg = sb_.tile([P, P], BF16, tag="agg")
                    nc.scalar.mul(agg[:], psA[w][:], inv_t[:, w:w + 1])
                    pt = ep_.tile([P, P], BF16, tag="T")
                    nc.tensor.transpose(out=pt[:], in_=agg[:],
                                        identity=ident_b[:])
                    aggT = sb_.tile([P, P], BF16, tag="aggT")
                    nc.vector.tensor_copy(out=aggT[:], in_=pt[:])
                    pb = ep_.tile([m_out, P], F32, tag="B")
                    nc.tensor.matmul(pb[:], lhsT=wl[:], rhs=aggT[:],
                                     start=True, stop=False)
                    nc.tensor.matmul(pb[:], lhsT=wr[:],
                                     rhs=dense_rhs[:, wc:wc + P],
                                     start=False, stop=True)
                    out_sink(w, pb, bias_t)
                if on_chunk_done is not None and (g + 1) % gpc == 0:
                    on_chunk_done(g // gpc)

        # ---- layer 1 ----
        def sink1(w, pb, bias_t):
            wc = w * P
            nc.scalar.activation(hT[:, wc:wc + P], pb[:],
                                 mybir.ActivationFunctionType.Relu,
                                 bias=bias_t[:], scale=1.0)
            pc = skp.tile([P, P], BF16, tag="C")
            nc.tensor.transpose(out=pc[:], in_=hT[:, wc:wc + P],
                                identity=ident_b[:])
            hrow = sko.tile([P, P], BF16, tag="hrow")
            nc.vector.tensor_copy(out=hrow[:], in_=pc[:])
            q = w // WQ
            c0 = wc - q * CH_SH
            nc.sync.dma_start(out=h_in_c[q][c0:c0 + P, :], in_=hrow[:])

        def chunk_done1(q):
            if not AG_END:
                issue_ag(h_in_c[q][:], h_full_c[q][:])

        sage_layer(x_full_c, xT, wt["wl1t"], wt["wr1t"], bl1_t,
                   m_out=P, out_sink=sink1,
                   on_chunk_done=chunk_done1)
        if AG_END:
            for q2 in range(Q):
                issue_ag(h_in_c[q2][:], h_full_c[q2][:])

        # ---- layer 2 ----
        def sink2(w, pb, bias_t):
            wc = w * P
            oT = sko.tile([NCLS, P], F32, tag="oT")
            nc.scalar.activation(oT[:], pb[:],
                                 mybir.ActivationFunctionType.Identity,
                                 bias=bias_t[:], scale=1.0)
            pc = skp.tile([P, NCLS], F32, tag="C")
            nc.tensor.matmul(pc[:], lhsT=oT[:], rhs=ident_f[:NCLS, :NCLS],
                             is_transpose=True)
            orow = sko.tile([P, NCLS], F32, tag="orow")
            nc.vector.tensor_copy(out=orow[:], in_=pc[:])
            nc.sync.dma_start(out=out_d[wc:wc + P, :], in_=orow[:])

        sage_layer(h_full_c, hT, wt["wl2t"], wt["wr2t"], bl2_t,
                   m_out=NCLS, out_sink=sink2)

        for pool in (sko, skp, sb_, ep_, ap_, ohp, gp, stage, cpool):
            pool.release()

    nc.compile()
    return nc


def make_inputs(cfg: Cfg, s: Schedule, x, Wl1, bl1, Wr1, Wl2, bl2, Wr2):
    """Per-core in_maps for run_bass_kernel_spmd."""
    C, SH, SHP, W, NCLS = cfg.C, cfg.SH, cfg.SHP, cfg.W, cfg.n_cls
    iota = np.tile(np.arange(128, dtype=np.float32), (128, 1))
    maps = []
    for c in range(C):
        xo = np.zeros((SHP, P), np.float32)
        xo[:SH] = x[c * SH:(c + 1) * SH]
        maps.append({
            "x_own": xo,
            "idx16": s.idx16[c],
            "dstloc": s.dstloc[c],
            "invdeg": s.invdeg_t[c],
            "iota": iota,
            "wl1t": np.ascontiguousarray(Wl1.T.astype(np.float32)),
            "wr1t": np.ascontiguousarray(Wr1.T.astype(np.float32)),
            "wl2t": np.ascontiguousarray(Wl2.T.astype(np.float32)),
            "wr2t": np.ascontiguousarray(Wr2.T.astype(np.float32)),
            "bl1": bl1.astype(np.float32).reshape(P, 1),
            "bl2": bl2.astype(np.float32).reshape(NCLS, 1),
        })
    return maps


def prepare(cfg: Cfg, x, edge_index, Wl1, bl1, Wr1, Wl2, bl2, Wr2):
    x = np.asarray(x, np.float32)
    ei = np.asarray(edge_index, np.int64)
    src, dst = ei[0], ei[1]
    deg = np.bincount(dst, minlength=cfg.N).astype(np.float32)
    s = build_schedule(cfg, src, dst, deg)
    maps = make_inputs(cfg, s, x, Wl1, bl1, Wr1, Wl2, bl2, Wr2)
    return s, maps


def run(x, edge_index, Wl1, bl1, Wr1, Wl2, bl2, Wr2, cfg=None, **spmd_kwargs):
    from concourse.bass_utils import run_bass_kernel_spmd
    cfg = cfg or Cfg(MAX_IDX=int(os.environ.get("K_MAX_IDX", "1024")))  # 2048 overflows the SWDGE ring -> device hang
    s, maps = prepare(cfg, x, edge_index, Wl1, bl1, Wr1, Wl2, bl2, Wr2)
    nc = build_program(cfg, s)
    res = run_bass_kernel_spmd(nc, maps, core_ids=list(range(cfg.C)),
                               **spmd_kwargs)
    out = np.concatenate([res.results[c]["out"][:cfg.SH]
                          for c in range(cfg.C)], axis=0)
    return out.astype(np.float32), res


def kernel(x, edge_index, Wl1, bl1, Wr1, Wl2, bl2, Wr2):
    out, _ = run(x, edge_index, Wl1, bl1, Wr1, Wl2, bl2, Wr2)
    return out
